# revision 5
# baseline (speedup 1.0000x reference)
"""Trainium2 Bass kernel for nn_ContinuumMemoryCell (scatter_memory).

Data-parallel over batch B across 8 NeuronCores. Device does the three
B-sized matmuls (error = x @ (V_w.T - M), y_pred = x @ M, and the Hebbian
partial dp_i = error_i.T @ x_i) plus the fused elementwise output
out = y_pred + mix * error. Everything O(D*H) or smaller (gate vectors,
sigmoid means, the final new_M AXPY, shard stitching) happens on host.
"""

import sys

if "/opt/trn_rl_repo" not in sys.path:
    sys.path.insert(0, "/opt/trn_rl_repo")

import numpy as np
import ml_dtypes

B, D, H = 16384, 1024, 1024
NCORES = 8
BL = B // NCORES          # 2048 batch rows per core
P = 128                   # partitions
NB = BL // P              # 16 b-tiles per core
NK = D // P               # 8 k-tiles (contraction over d)
NH = H // P               # 8 h-tiles (dp output rows)
FD = 512                  # matmul moving free-dim (one PSUM bank of f32)

_CACHE = {}


def _build():
    """Build + compile the SPMD Bass program (once per process)."""
    if "nc" in _CACHE:
        return _CACHE["nc"]

    import concourse.bacc as bacc
    import concourse.mybir as mybir
    import concourse.tile as tile

    bf16 = mybir.dt.bfloat16
    f32 = mybir.dt.float32

    nc = bacc.Bacc("TRN2", target_bir_lowering=False, debug=False,
                   num_devices=NCORES)

    xT_d = nc.dram_tensor("xT", [D, BL], bf16, kind="ExternalInput")
    xn_d = nc.dram_tensor("xn", [BL, D], bf16, kind="ExternalInput")
    we_d = nc.dram_tensor("we", [D, H], bf16, kind="ExternalInput")
    mm_d = nc.dram_tensor("mm", [D, D], bf16, kind="ExternalInput")
    mx_d = nc.dram_tensor("mx", [P, NB], f32, kind="ExternalInput")
    out_d = nc.dram_tensor("out", [BL, H], f32, kind="ExternalOutput")
    dp_d = nc.dram_tensor("dp", [H, D], f32, kind="ExternalOutput")

    with tile.TileContext(nc) as tc:
        with (
            tc.tile_pool(name="big", bufs=1) as big,
            tc.tile_pool(name="work", bufs=4) as work,
            tc.tile_pool(name="ps", bufs=4, space="PSUM") as ps,
        ):
            # Resident SBUF tensors, one tile per 128-row chunk so Tile
            # tracks DMA->matmul deps at chunk granularity.
            xT_sb = [big.tile([P, BL], bf16, tag=f"xT{k}", name=f"xT{k}")
                     for k in range(NK)]
            we_sb = [big.tile([P, H], bf16, tag=f"we{k}", name=f"we{k}")
                     for k in range(NK)]
            mm_sb = [big.tile([P, D], bf16, tag=f"mm{k}", name=f"mm{k}")
                     for k in range(NK)]
            xn_sb = [big.tile([P, D], bf16, tag=f"xn{i}", name=f"xn{i}")
                     for i in range(NB)]
            err_sb = [big.tile([P, H], bf16, tag=f"err{i}", name=f"err{i}")
                      for i in range(NB)]
            mx_sb = big.tile([P, NB], f32, tag="mx", name="mx")

            nc.sync.dma_start(mx_sb[:], mx_d[:])
            for k in range(NK):
                nc.sync.dma_start(xT_sb[k][:], xT_d[k * P:(k + 1) * P, :])
                nc.sync.dma_start(we_sb[k][:], we_d[k * P:(k + 1) * P, :])
                nc.sync.dma_start(mm_sb[k][:], mm_d[k * P:(k + 1) * P, :])
            for i in range(NB):
                nc.sync.dma_start(xn_sb[i][:], xn_d[i * P:(i + 1) * P, :])

            # Phase 1: per b-tile, err = x @ We and y = x @ M with the same
            # stationary xT tile; fused epilogue out = err*mix + y.
            for i in range(NB):
                pe = ps.tile([P, H], f32, tag="acc", name=f"pe{i}")
                py = ps.tile([P, D], f32, tag="acc", name=f"py{i}")
                for k in range(NK):
                    lhs = xT_sb[k][:, i * P:(i + 1) * P]
                    st, sp = (k == 0), (k == NK - 1)
                    for h2 in range(2):
                        nc.tensor.matmul(pe[:, h2 * FD:(h2 + 1) * FD], lhs,
                                         we_sb[k][:, h2 * FD:(h2 + 1) * FD],
                                         start=st, stop=sp)
                    for h2 in range(2):
                        nc.tensor.matmul(py[:, h2 * FD:(h2 + 1) * FD], lhs,
                                         mm_sb[k][:, h2 * FD:(h2 + 1) * FD],
                                         start=st, stop=sp)
                nc.any.tensor_copy(err_sb[i][:], pe[:])
                for h2 in range(2):
                    o = work.tile([P, FD], f32, tag="o", name=f"o{i}_{h2}")
                    nc.vector.scalar_tensor_tensor(
                        o[:], err_sb[i][:, h2 * FD:(h2 + 1) * FD],
                        mx_sb[:, i:i + 1],
                        py[:, h2 * FD:(h2 + 1) * FD],
                        mybir.AluOpType.mult, mybir.AluOpType.add)
                    nc.sync.dma_start(
                        out_d[i * P:(i + 1) * P, h2 * FD:(h2 + 1) * FD], o[:])

            # Phase 2: dp[h-tile] = sum_b err[b, h-tile].T @ x[b, :]
            for t in range(NH):
                pd = ps.tile([P, D], f32, tag="acc", name=f"pd{t}")
                for i in range(NB):
                    lhs = err_sb[i][:, t * P:(t + 1) * P]
                    st, sp = (i == 0), (i == NB - 1)
                    for h2 in range(2):
                        nc.tensor.matmul(pd[:, h2 * FD:(h2 + 1) * FD], lhs,
                                         xn_sb[i][:, h2 * FD:(h2 + 1) * FD],
                                         start=st, stop=sp)
                dpt = work.tile([P, D], f32, tag="dpt", name=f"dpt{t}")
                nc.any.tensor_copy(dpt[:], pd[:])
                nc.sync.dma_start(dp_d[t * P:(t + 1) * P, :], dpt[:])

    nc.compile()
    _CACHE["nc"] = nc
    return nc


def _prepare(inputs):
    """Host-side preprocessing: shard + dtype-convert + gate math."""
    x = np.asarray(inputs["x"], np.float32)
    V_w = np.asarray(inputs["V_w"], np.float32)
    M = np.asarray(inputs["M"], np.float32)
    fg_w = np.asarray(inputs["fg_w"], np.float32)
    fg_b = np.asarray(inputs["fg_b"], np.float32)
    ug_w = np.asarray(inputs["ug_w"], np.float32)
    ug_b = np.asarray(inputs["ug_b"], np.float32)
    sm_w = np.asarray(inputs["sm_w"], np.float32)
    sm_b = np.asarray(inputs["sm_b"], np.float32)

    bf16 = ml_dtypes.bfloat16
    VT = V_w.T                                # [D, H]
    we = np.ascontiguousarray(VT - M).astype(bf16)
    mm = np.ascontiguousarray(M).astype(bf16)

    # v @ a == x @ (V_w.T @ a): collapse each gate to one D-vector on x.
    c_f = VT @ fg_w[0, :H] + fg_w[0, H:]
    c_u = VT @ ug_w[0, :H] + ug_w[0, H:]
    c_m = VT @ sm_w[0]
    logits = x @ np.stack([c_f, c_u, c_m], axis=1)       # [B, 3]
    sig = 1.0 / (1.0 + np.exp(-(logits + np.array([fg_b[0], ug_b[0], sm_b[0]]))))
    fmean = float(sig[:, 0].mean())
    umean = float(sig[:, 1].mean())
    mix = sig[:, 2].astype(np.float32)                   # [B]

    xb = x.astype(bf16)
    in_maps = []
    for i in range(NCORES):
        s = slice(i * BL, (i + 1) * BL)
        in_maps.append({
            "xT": np.ascontiguousarray(xb[s].T),
            "xn": np.ascontiguousarray(xb[s]),
            "we": we,
            "mm": mm,
            "mx": np.ascontiguousarray(mix[s].reshape(NB, P).T),
        })
    return in_maps, M, fmean, umean


def _finish(results, M, fmean, umean):
    out = np.concatenate([results[i]["out"] for i in range(NCORES)], axis=0)
    delta = results[0]["dp"].astype(np.float64)
    for i in range(1, NCORES):
        delta += results[i]["dp"]
    delta_mean = (delta / B).astype(np.float32)
    new_M = fmean * M + umean * 0.1 * delta_mean
    return out.astype(np.float32), new_M.astype(np.float32)


def _run(inputs, trace=False, trace_kwargs=None):
    from concourse.bass_utils import run_bass_kernel_spmd

    nc = _build()
    in_maps, M, fmean, umean = _prepare(inputs)
    res = run_bass_kernel_spmd(nc, in_maps, core_ids=list(range(NCORES)),
                               trace=trace, **(trace_kwargs or {}))
    return _finish(res.results, M, fmean, umean), res


def kernel(**inputs):
    (out, new_M), _ = _run(inputs)
    return out, new_M
